# revision 1
# baseline (speedup 1.0000x reference)
"""Fused decoder attention block (self-attn + cross-attn + MLP) on 8 TRN2 NeuronCores.

Sharding: data-parallel over batch (B=16 -> 2 per core). No collectives.
Layout: feature-major residual stream xT [D, n_tok] per core; LN stats via
PE ones-matmuls (float32r); attention computes scores^T [S, T] so exp(scores)
is directly the lhsT of the PV matmul; softmax normalization via a ones-column
appended to V (row 64 of the PV psum = row sums). q/k/v round-trip through
DRAM scratch (each element is consumed exactly once by attention).

Self-contained: hardcodes all shapes; only imports the system bass stack.
"""
import sys

sys.path.insert(0, "/opt/trn_rl_repo")

import numpy as np
import ml_dtypes

import concourse.tile as tile
from concourse import bacc, mybir
from concourse import bass_utils

F32 = mybir.dt.float32
F32R = mybir.dt.float32r
BF16 = mybir.dt.bfloat16
AF = mybir.ActivationFunctionType
ALU = mybir.AluOpType
BF16NP = ml_dtypes.bfloat16

D = 1024
H = 16
HD = 64
T = 512
S = 1024
B = 16
NCORES = 8
BPC = B // NCORES            # batches per core = 2
N = T * BPC                  # x tokens per core = 1024
M = S * BPC                  # hidden tokens per core = 2048
DFF = 4 * D
KT = D // 128                # 8 k-tiles over D
EPS = 1e-5
GELU_A = 1.702


# ---------------------------------------------------------------------------
# device program pieces
# ---------------------------------------------------------------------------

def _emit_ln(nc, po, xbuf, ones32):
    """LayerNorm (affine folded into weights) of feature-major xbuf
    [128, KT, N] f32 -> returns new bf16 tile h [128, KT, N].

    Processed in 512-token chunks so chunk 0's output unblocks downstream
    matmuls while chunk 1's stats/chain still run."""
    h = po["res"].tile([128, KT, N], BF16, tag="h")
    for ch in range(N // 512):
        sl = slice(ch * 512, (ch + 1) * 512)
        ps_s = po["psum_sc"].tile([1, 512], F32, tag="sc")
        ps_q = po["psum_sc"].tile([1, 512], F32, tag="sc")
        for kt in range(KT):
            xb = po["work"].tile([128, 512], BF16, tag="xbc")
            nc.gpsimd.tensor_copy(xb[:], xbuf[:, kt, sl])
            x2c = po["work"].tile([128, 512], BF16, tag="x2c")
            nc.gpsimd.tensor_tensor(x2c[:], xbuf[:, kt, sl], xbuf[:, kt, sl],
                                    ALU.mult)
            nc.tensor.matmul(ps_s[:], ones32[:], xb[:],
                             start=(kt == 0), stop=(kt == KT - 1))
            nc.tensor.matmul(ps_q[:], ones32[:], x2c[:],
                             start=(kt == 0), stop=(kt == KT - 1))

        m = po["small"].tile([1, 512], F32, tag="m")
        var = po["small"].tile([1, 512], F32, tag="var")
        rstd = po["small"].tile([1, 512], F32, tag="rstd")
        nc.vector.tensor_scalar_mul(m[:], ps_s[:], 1.0 / D)
        mm = po["small"].tile([1, 512], F32, tag="mm")
        nc.vector.tensor_tensor(mm[:], m[:], m[:], ALU.mult)
        # var = ps_q/D - m^2 + eps
        nc.vector.scalar_tensor_tensor(var[:], ps_q[:], 1.0 / D, mm[:],
                                       ALU.mult, ALU.subtract)
        nc.vector.tensor_scalar_add(var[:], var[:], EPS)
        # rstd = exp(-0.5*ln(var)); Ln & Exp share one ACT table set
        nc.scalar.activation(var[:], var[:], AF.Ln, bias=0.0)
        nc.scalar.activation(rstd[:], var[:], AF.Exp, scale=-0.5)
        nmrs = po["small"].tile([1, 512], F32, tag="mm")
        nc.vector.scalar_tensor_tensor(nmrs[:], m[:], -1.0, rstd[:],
                                       ALU.mult, ALU.mult)

        a_b = po["small"].tile([128, 512], F32, tag="Ab")
        b_b = po["small"].tile([128, 512], F32, tag="Bb")
        nc.gpsimd.partition_broadcast(a_b[:], rstd[0:1, :])
        nc.gpsimd.partition_broadcast(b_b[:], nmrs[0:1, :])

        for kt in range(KT):
            nc.vector.tensor_tensor(h[:, kt, sl], xbuf[:, kt, sl], a_b[:],
                                    ALU.mult)
            nc.vector.tensor_tensor(h[:, kt, sl], h[:, kt, sl], b_b[:],
                                    ALU.add)
    return h


def _emit_fm_proj(nc, po, w_ap, n_ot, kt_count, rhs3, tok_sl, out_cb, wtag):
    """Feature-major projection: for ot: psum[128,512] = sum_kt W[:,ot,kt].T @ rhs.

    w_ap: dram [128, n_ot, kt_count, 128]; rhs3: sbuf [128, kt_count, ntok];
    tok_sl: slice of 512 tokens. out_cb(ot, psum)."""
    for ot in range(n_ot):
        wst = po["w"].tile([128, kt_count, 128], BF16, tag=wtag)
        nc.sync.dma_start(wst[:], w_ap[:, ot])
        ps = po["psum_pr"].tile([128, 512], F32, tag="proj")
        for kt in range(kt_count):
            nc.tensor.matmul(ps[:], wst[:, kt], rhs3[:, kt, tok_sl],
                             start=(kt == 0), stop=(kt == kt_count - 1))
        out_cb(ot, ps)


def _stage_to_dram(nc, po, ps, dram_ap, bias_ap):
    """psum -> bf16 staging tile (+bias) -> DMA to dram_ap ([128, 512]-shaped)."""
    stg = po["work"].tile([128, 512], BF16, tag="stg")
    if bias_ap is None:
        nc.vector.tensor_copy(stg[:], ps[:])
    else:
        nc.vector.tensor_scalar_add(stg[:], ps[:], bias_ap)
    nc.sync.dma_start(dram_ap, stg[:])
    return stg


def _emit_attention(nc, po, q_dr, k_dr, v_dr, ctxT, s_len):
    """Streaming attention: per (b, head-pair) load q/k strips; per head load
    V-strip, scores^T = K@Q^T -> exp -> PV with ones-column -> normalized ctx^T."""
    n_s = s_len // 128
    for b in range(BPC):
        for hp in range(H // 2):
            qp = po["strm"].tile([128, T], BF16, tag="qs")
            kp = po["strm"].tile([128, s_len], BF16, tag="ks")
            for j in range(2):
                off = 64 * j
                nc.sync.dma_start(qp[off:off + 64, :],
                                  q_dr[off:off + 64, hp, b * T:(b + 1) * T])
                nc.sync.dma_start(kp[off:off + 64, :],
                                  k_dr[off:off + 64, hp,
                                       b * s_len:(b + 1) * s_len])
            for j in range(2):
                h = hp * 2 + j
                off = 64 * j
                vst = po["strm"].tile([128, n_s, 65], BF16, tag="vs")
                nc.vector.memset(vst[:, :, 64:65], 1.0)
                nc.sync.dma_start(vst[:, :, 0:64],
                                  v_dr[:, h, b * n_s:(b + 1) * n_s, :])
                etiles = []
                for sp in range(n_s // 2):
                    sc = po["psum_sc"].tile([128, 1024], F32, tag="sc")
                    for jj in range(2):
                        s = sp * 2 + jj
                        nc.tensor.matmul(
                            sc[:, jj * 512:(jj + 1) * 512],
                            kp[off:off + 64, s * 128:(s + 1) * 128],
                            qp[off:off + 64, :], start=True, stop=True)
                    e = po["ew"].tile([128, 1024], BF16, tag="E")
                    nc.scalar.activation(e[:], sc[:], AF.Exp)
                    etiles.append(e)
                ctx = po["psum_ctx"].tile([65, 512], F32, tag="ctx")
                for s in range(n_s):
                    nc.tensor.matmul(
                        ctx[:], vst[:, s, :],
                        etiles[s // 2][:, (s % 2) * 512:(s % 2 + 1) * 512],
                        start=(s == 0), stop=(s == n_s - 1))
                # softmax denominators live in psum row 64: copy the row out,
                # DMA it down to partition 0 (gpsimd broadcast reads the
                # physical partition 0 on HW), broadcast, then divide rows 0-63.
                rin = po["work"].tile([65, 512], F32, tag="rinv")
                nc.vector.tensor_copy(rin[64:65, :], ctx[64:65, :])
                ri0 = po["work"].tile([1, 512], F32, tag="ri0")
                nc.sync.dma_start(ri0[0:1, :], rin[64:65, :])
                rb = po["work"].tile([64, 512], F32, tag="rb")
                nc.gpsimd.partition_broadcast(rb[:, :], ri0[0:1, :])
                nc.vector.reciprocal(rb[:, :], rb[:, :])
                dst_sl = slice(b * T, (b + 1) * T)
                if j == 0:
                    nc.vector.tensor_tensor(ctxT[0:64, hp, dst_sl],
                                            ctx[0:64, :], rb[:, :], ALU.mult)
                else:
                    tmp = po["work"].tile([64, 512], BF16, tag="ctxodd")
                    nc.vector.tensor_tensor(tmp[:], ctx[0:64, :], rb[:, :],
                                            ALU.mult)
                    nc.sync.dma_start(ctxT[64:128, hp, dst_sl], tmp[:])


def build_program(use_bias):
    nc = bacc.Bacc("TRN2", target_bir_lowering=False, debug=False,
                   enable_asserts=False, num_devices=NCORES)

    def din(name, shape, dt=BF16):
        return nc.dram_tensor(name, shape, dt, kind="ExternalInput").ap()

    xT_d = din("xT", [128, KT, N], F32)
    hT_d = din("hT", [128, KT, M])
    wqk_d = din("wqk", [128, 16, KT, 128])        # q:0-7, k:8-15
    wvsa_d = din("wvsa", [128, KT, D])            # rhs layout for token-major V
    wosa_d = din("wosa", [128, 8, KT, 128])
    wqca_d = din("wqca", [128, 8, KT, 128])
    wkca_d = din("wkca", [128, 8, KT, 128])
    wvca_d = din("wvca", [128, KT, D])
    wfc_d = din("wfc", [128, 32, KT, 128])
    wproj_d = din("wproj", [128, 8, 32, 128])
    wo_ca_d = din("woca", [128, 8, KT, 128])
    any_bias = any(use_bias.values())
    if any_bias:
        bfm_d = din("bias_fm", [128, 96], F32)
        brow_d = din("bias_rows", [1, 2 * D], F32)
    outT_d = nc.dram_tensor("outT", [128, KT, N], F32,
                            kind="ExternalOutput").ap()

    from contextlib import ExitStack
    with tile.TileContext(nc) as tc, ExitStack() as ctx:
        po = {}
        po["res"] = ctx.enter_context(tc.tile_pool(name="res", bufs=1))
        po["w"] = ctx.enter_context(tc.tile_pool(name="w", bufs=3))
        po["small"] = ctx.enter_context(tc.tile_pool(name="small", bufs=1))
        po["work"] = ctx.enter_context(tc.tile_pool(name="work", bufs=3))
        po["strm"] = ctx.enter_context(tc.tile_pool(name="strm", bufs=2))
        po["ew"] = ctx.enter_context(tc.tile_pool(name="ew", bufs=3))
        po["dram"] = ctx.enter_context(
            tc.tile_pool(name="dram", bufs=1, space="DRAM"))
        po["psum_pr"] = ctx.enter_context(
            tc.tile_pool(name="psum_pr", bufs=2, space="PSUM"))
        po["psum_sc"] = ctx.enter_context(
            tc.tile_pool(name="psum_sc", bufs=2, space="PSUM"))
        po["psum_ctx"] = ctx.enter_context(
            tc.tile_pool(name="psum_ctx", bufs=2, space="PSUM"))

        ones32 = po["res"].tile([128, 1], BF16, tag="ones")
        nc.vector.memset(ones32[:], 1.0)
        if any_bias:
            bfm = po["res"].tile([128, 96], F32, tag="bfm")
            nc.sync.dma_start(bfm[:], bfm_d[:])
            brow = po["res"].tile([1, 2 * D], F32, tag="brow")
            nc.sync.dma_start(brow[:], brow_d[:])

        def bcol(c):
            return bfm[:, c:c + 1] if any_bias else None

        xbuf = po["res"].tile([128, KT, N], F32, tag="xbuf")
        for ch in range(N // 512):
            sl = slice(ch * 512, (ch + 1) * 512)
            nc.sync.dma_start(xbuf[:, :, sl], xT_d[:, :, sl])

        def vrow_bcast(col0):
            t = po["small"].tile([128, D], F32, tag="vbias")
            nc.gpsimd.partition_broadcast(t[:], brow[0:1, col0:col0 + D])
            return t

        def emit_v_proj(h3, wv_d, v_dr, n_tok, vb):
            """Token-major V projection: v[tok, dv] staged to v_dr[p,h,sub,64]."""
            for ch in range(2):           # dv chunks of 512 = 8 heads
                wvc = po["w"].tile([128, KT, 512], BF16, tag="wbig")
                nc.sync.dma_start(wvc[:], wv_d[:, :, ch * 512:(ch + 1) * 512])
                for tt in range(n_tok // 128):
                    tsl = slice(tt * 128, (tt + 1) * 128)
                    ps = po["psum_pr"].tile([128, 512], F32, tag="proj")
                    for kt in range(KT):
                        nc.tensor.matmul(ps[:], h3[:, kt, tsl], wvc[:, kt],
                                         start=(kt == 0), stop=(kt == KT - 1))
                    stg = po["work"].tile([128, 512], BF16, tag="stg")
                    if vb is None:
                        nc.vector.tensor_copy(stg[:], ps[:])
                    else:
                        nc.vector.tensor_tensor(
                            stg[:], ps[:], vb[:, ch * 512:(ch + 1) * 512],
                            ALU.add)
                    nc.sync.dma_start(
                        v_dr[:, ch * 8:(ch + 1) * 8, tt, :],
                        stg[:].rearrange("p (h e) -> p h e", e=64))

        # ---- stage 1: LN1 + self-attention -------------------------------
        h1 = _emit_ln(nc, po, xbuf, ones32)

        q_s = po["dram"].tile([128, 8, N], BF16, tag="q_s")
        k_s = po["dram"].tile([128, 8, N], BF16, tag="k_s")
        v_s = po["dram"].tile([128, 16, N // 128, 64], BF16, tag="v_s")

        for bch in range(2):              # token chunks of 512 (= batch b)
            tsl = slice(bch * 512, (bch + 1) * 512)

            def qk_cb(ot, ps, _tsl=tsl):
                dst = q_s if ot < 8 else k_s
                o = ot % 8
                bc = bcol((0 if ot < 8 else 8) + o) if use_bias["qk_sa"] else None
                _stage_to_dram(nc, po, ps, dst[:, o, _tsl], bc)
            _emit_fm_proj(nc, po, wqk_d, 16, KT, h1, tsl, qk_cb, "wst8")
        vb = vrow_bcast(0) if use_bias["v_sa"] else None
        emit_v_proj(h1, wvsa_d, v_s, N, vb)

        ctxT = po["res"].tile([128, 8, N], BF16, tag="ctxT")
        _emit_attention(nc, po, q_s, k_s, v_s, ctxT, T)

        def emit_out_proj(w_d, src, bias_base, flag):
            for bch in range(2):
                tsl = slice(bch * 512, (bch + 1) * 512)

                def cb(ot, ps, _tsl=tsl):
                    if flag:
                        scr = po["work"].tile([128, 512], F32, tag="rescr")
                        nc.vector.tensor_scalar_add(scr[:], ps[:],
                                                    bcol(bias_base + ot))
                        nc.vector.tensor_tensor(xbuf[:, ot, _tsl], scr[:],
                                                xbuf[:, ot, _tsl], ALU.add)
                    else:
                        nc.vector.tensor_tensor(xbuf[:, ot, _tsl], ps[:],
                                                xbuf[:, ot, _tsl], ALU.add)
                _emit_fm_proj(nc, po, w_d, 8, KT, src, tsl, cb, "wst8")

        emit_out_proj(wosa_d, ctxT, 16, use_bias["o_sa"])

        # ---- stage 2: LN2 + cross-attention ------------------------------
        h2 = _emit_ln(nc, po, xbuf, ones32)

        q_c = po["dram"].tile([128, 8, N], BF16, tag="q_c")
        k_c = po["dram"].tile([128, 8, M], BF16, tag="k_c")
        v_c = po["dram"].tile([128, 16, M // 128, 64], BF16, tag="v_c")

        for bch in range(2):
            tsl = slice(bch * 512, (bch + 1) * 512)

            def q2_cb(ot, ps, _tsl=tsl):
                bc = bcol(24 + ot) if use_bias["q_ca"] else None
                _stage_to_dram(nc, po, ps, q_c[:, ot, _tsl], bc)
            _emit_fm_proj(nc, po, wqca_d, 8, KT, h2, tsl, q2_cb, "wst8")

        # cross K/V from hidden_states, streamed in 512-token chunks
        for hch in range(M // 512):
            hsl = slice(hch * 512, (hch + 1) * 512)
            hTc = po["w"].tile([128, KT, 512], BF16, tag="wbig")
            nc.sync.dma_start(hTc[:], hT_d[:, :, hsl])
            for ot in range(8):
                wst = po["w"].tile([128, KT, 128], BF16, tag="wst8")
                nc.sync.dma_start(wst[:], wkca_d[:, ot])
                ps = po["psum_pr"].tile([128, 512], F32, tag="proj")
                for kt in range(KT):
                    nc.tensor.matmul(ps[:], wst[:, kt], hTc[:, kt],
                                     start=(kt == 0), stop=(kt == KT - 1))
                bc = bcol(32 + ot) if use_bias["k_ca"] else None
                _stage_to_dram(nc, po, ps, k_c[:, ot, hsl], bc)
            vbc = vrow_bcast(D) if use_bias["v_ca"] else None
            for ch in range(2):
                wvc = po["w"].tile([128, KT, 512], BF16, tag="wbig")
                nc.sync.dma_start(wvc[:], wvca_d[:, :, ch * 512:(ch + 1) * 512])
                for tt in range(4):
                    sub = hch * 4 + tt
                    tsl2 = slice(hch * 512 + tt * 128, hch * 512 + (tt + 1) * 128)
                    ps = po["psum_pr"].tile([128, 512], F32, tag="proj")
                    for kt in range(KT):
                        nc.tensor.matmul(ps[:], hTc[:, kt, tt * 128:(tt + 1) * 128],
                                         wvc[:, kt], start=(kt == 0),
                                         stop=(kt == KT - 1))
                    stg = po["work"].tile([128, 512], BF16, tag="stg")
                    if vbc is None:
                        nc.vector.tensor_copy(stg[:], ps[:])
                    else:
                        nc.vector.tensor_tensor(
                            stg[:], ps[:], vbc[:, ch * 512:(ch + 1) * 512],
                            ALU.add)
                    nc.sync.dma_start(
                        v_c[:, ch * 8:(ch + 1) * 8, sub, :],
                        stg[:].rearrange("p (h e) -> p h e", e=64))

        ctx2 = po["res"].tile([128, 8, N], BF16, tag="ctxT")
        _emit_attention(nc, po, q_c, k_c, v_c, ctx2, S)
        emit_out_proj(wo_ca_d, ctx2, 40, use_bias["o_ca"])

        # ---- stage 3: LN3 + MLP (token-chunked) --------------------------
        h3 = _emit_ln(nc, po, xbuf, ones32)

        for bch in range(2):
            tsl = slice(bch * 512, (bch + 1) * 512)
            gT = po["res"].tile([128, 32, 512], BF16, tag="gT")

            def fc_cb(ot, ps, _g=gT):
                sg = po["work"].tile([128, 512], BF16, tag="sg")
                if use_bias["fc"]:
                    scr = po["work"].tile([128, 512], F32, tag="rescr")
                    nc.vector.tensor_scalar_add(scr[:], ps[:], bcol(48 + ot))
                    nc.scalar.activation(sg[:], scr[:], AF.Sigmoid,
                                         scale=GELU_A)
                    nc.vector.tensor_tensor(_g[:, ot], scr[:], sg[:], ALU.mult)
                else:
                    nc.scalar.activation(sg[:], ps[:], AF.Sigmoid, scale=GELU_A)
                    nc.vector.tensor_tensor(_g[:, ot], ps[:], sg[:], ALU.mult)
            _emit_fm_proj(nc, po, wfc_d, 32, KT, h3, tsl, fc_cb, "wst8")

            def proj_cb(ot, ps, _tsl=tsl):
                if use_bias["proj"]:
                    scr = po["work"].tile([128, 512], F32, tag="rescr")
                    nc.vector.tensor_scalar_add(scr[:], ps[:], bcol(88 + ot))
                    nc.vector.tensor_tensor(xbuf[:, ot, _tsl], scr[:],
                                            xbuf[:, ot, _tsl], ALU.add)
                else:
                    nc.vector.tensor_tensor(xbuf[:, ot, _tsl], ps[:],
                                            xbuf[:, ot, _tsl], ALU.add)
                nc.sync.dma_start(outT_d[:, ot, _tsl], xbuf[:, ot, _tsl])
            _emit_fm_proj(nc, po, wproj_d, 8, 32, gT, slice(0, 512),
                          proj_cb, "wbig")

    nc.compile()
    return nc


# ---------------------------------------------------------------------------
# host side
# ---------------------------------------------------------------------------

def _tile4(w):
    """[Din, Dout] -> [128, Dout/128, Din/128, 128] (p, ot, kt, o)."""
    din, dout = w.shape
    return np.ascontiguousarray(
        w.reshape(din // 128, 128, dout // 128, 128).transpose(1, 2, 0, 3))


def _rhs_tiled(w):
    """[Din, Dout] -> [128, Din/128, Dout] (p, kt, o)."""
    din, dout = w.shape
    return np.ascontiguousarray(
        w.reshape(din // 128, 128, dout).transpose(1, 0, 2))


def _fm_cols(b):
    """[Dout] -> [128, Dout/128] (p, ot)."""
    return np.ascontiguousarray(b.reshape(-1, 128).T)


def _prep_host(inputs):
    f32 = np.float32
    g = {k: np.asarray(v, f32) for k, v in inputs.items()}
    x, hs = g["x"], g["hidden_states"]
    scale = f32(1.0 / np.sqrt(HD))

    wq, wk, wv = np.split(g["sa_in_w"], 3, axis=0)
    bq, bk, bv = np.split(g["sa_in_b"], 3)
    wq_e = (wq * g["ln1_g"][None, :]) * scale
    bq_e = (wq @ g["ln1_b"]) * scale + bq
    wk_e = wk * g["ln1_g"][None, :]
    bk_e = wk @ g["ln1_b"] + bk
    wv_e = wv * g["ln1_g"][None, :]
    bv_e = wv @ g["ln1_b"] + bv

    cq, ck, cv = np.split(g["ca_in_w"], 3, axis=0)
    cbq, cbk, cbv = np.split(g["ca_in_b"], 3)
    cq_e = (cq * g["ln2_g"][None, :]) * scale
    cbq_e = (cq @ g["ln2_b"]) * scale + cbq
    # k/v of cross-attn apply to raw hidden_states: no LN fold
    fc_e = g["fc_w"] * g["ln3_g"][None, :]
    fcb_e = g["fc_w"] @ g["ln3_b"] + g["fc_b"]

    wqk = np.concatenate([wq_e, wk_e], axis=0)     # [2D, D]
    nz = lambda a: bool(np.abs(a).max() > 0)
    use_bias = dict(
        qk_sa=nz(np.concatenate([bq_e, bk_e])), v_sa=nz(bv_e),
        o_sa=nz(g["sa_out_b"]), q_ca=nz(cbq_e), k_ca=nz(cbk), v_ca=nz(cbv),
        o_ca=nz(g["ca_out_b"]), fc=nz(fcb_e), proj=nz(g["proj_b"]),
    )

    bf = lambda a: np.ascontiguousarray(a.astype(BF16NP))
    weights = {
        "wqk": bf(_tile4(wqk.T)),
        "wvsa": bf(_rhs_tiled(wv_e.T)),
        "wosa": bf(_tile4(g["sa_out_w"].T)),
        "wqca": bf(_tile4(cq_e.T)),
        "wkca": bf(_tile4(ck.T)),
        "wvca": bf(_rhs_tiled(cv.T)),
        "woca": bf(_tile4(g["ca_out_w"].T)),
        "wfc": bf(_tile4(fc_e.T)),
        "wproj": bf(_tile4(g["proj_w"].T)),
    }
    if any(use_bias.values()):
        bfm = np.zeros((128, 96), f32)
        bfm[:, 0:8] = _fm_cols(bq_e)
        bfm[:, 8:16] = _fm_cols(bk_e)
        bfm[:, 16:24] = _fm_cols(g["sa_out_b"])
        bfm[:, 24:32] = _fm_cols(cbq_e)
        bfm[:, 32:40] = _fm_cols(cbk)
        bfm[:, 40:48] = _fm_cols(g["ca_out_b"])
        bfm[:, 48:80] = _fm_cols(fcb_e)
        bfm[:, 88:96] = _fm_cols(g["proj_b"])
        brow = np.zeros((1, 2 * D), f32)
        brow[0, 0:D] = bv_e
        brow[0, D:2 * D] = cbv
        weights["bias_fm"] = bfm
        weights["bias_rows"] = brow

    in_maps = []
    for c in range(NCORES):
        xs = x[:, 2 * c:2 * c + 2, :]              # [T, 2, D]
        xt = xs.transpose(2, 1, 0).reshape(KT, 128, N).transpose(1, 0, 2)
        hss = hs[:, 2 * c:2 * c + 2, :]
        ht = hss.transpose(2, 1, 0).reshape(KT, 128, M).transpose(1, 0, 2)
        im = dict(weights)
        im["xT"] = np.ascontiguousarray(xt.astype(f32))
        im["hT"] = bf(ht)
        in_maps.append(im)
    return in_maps, use_bias


def _unshard(results):
    out = np.empty((T, B, D), np.float32)
    for c in range(NCORES):
        r = np.asarray(results[c]["outT"])         # [128, KT, N]
        arr = r.transpose(1, 0, 2).reshape(D, BPC, T)
        out[:, 2 * c:2 * c + 2, :] = arr.transpose(2, 1, 0)
    return out


_cache = {}


def _get_program(key):
    if key not in _cache:
        _cache[key] = build_program(dict(key))
    return _cache[key]


def kernel(**inputs):
    in_maps, use_bias = _prep_host(inputs)
    nc = _get_program(tuple(sorted(use_bias.items())))
    res = bass_utils.run_bass_kernel_spmd(nc, in_maps,
                                          core_ids=list(range(NCORES)))
    return _unshard(res.results)


def kernel_traced(**inputs):
    """Like kernel() but with NTFF profiling; returns (out, exec_time_ns)."""
    import types
    import antenv  # noqa: F401
    if "antenv.axon_hooks" not in sys.modules:
        hooks = types.ModuleType("antenv.axon_hooks")
        hooks._hook = None
        hooks.set_axon_ntff_profile_hook = lambda h: setattr(hooks, "_hook", h)
        hooks.get_axon_ntff_profile_hook = lambda: hooks._hook
        sys.modules["antenv.axon_hooks"] = hooks
        try:
            import trn_agent_boot.trn_boot as _tb
            hooks._hook = _tb._ntff_profile_via_ctypes("/opt/axon/libaxon_pjrt.so")
        except Exception as e:  # pragma: no cover
            print("ntff hook unavailable:", e)
    in_maps, use_bias = _prep_host(inputs)
    nc = _get_program(tuple(sorted(use_bias.items())))
    res = bass_utils.run_bass_kernel_spmd(nc, in_maps,
                                          core_ids=list(range(NCORES)),
                                          trace=True)
    return _unshard(res.results), res.exec_time_ns



# revision 11
# speedup vs baseline: 1.0749x; 1.0749x over previous
"""Fused decoder attention block (self-attn + cross-attn + MLP) on 8 TRN2 NeuronCores.

Sharding: data-parallel over batch (B=16 -> 2 per core). No collectives.
v2 schedule: feature-major residual xT [D, n_tok]; q/k staged through DRAM
with contiguous tiles, v kept in SBUF (its relayout was the DMA-descriptor
hotspot); score matmuls for a head pair issued back-to-back so they run
row-tiled (64+64) concurrently in the PE array; attention (ACT-exp-bound) is
zipped at emission time with independent projection matmuls (cross-attn K
during self-attn, MLP of batch 0 during cross-attn of batch 1) so the PE
never idles; softmax denominators via a ones-column in V, normalized with
reciprocal_approx_fast.

Self-contained: hardcodes all shapes; only imports the system bass stack.
"""
import sys

sys.path.insert(0, "/opt/trn_rl_repo")

import numpy as np
import ml_dtypes

import concourse.tile as tile
from concourse import bacc, mybir
from concourse import bass_utils

F32 = mybir.dt.float32
BF16 = mybir.dt.bfloat16
AF = mybir.ActivationFunctionType
ALU = mybir.AluOpType
BF16NP = ml_dtypes.bfloat16

D = 1024
H = 16
HD = 64
T = 512
S = 1024
B = 16
NCORES = 8
BPC = B // NCORES            # batches per core = 2
N = T * BPC                  # x tokens per core = 1024
M = S * BPC                  # hidden tokens per core = 2048
DFF = 4 * D
KT = D // 128                # 8 k-tiles over D
EPS = 1e-5
GELU_A = 1.702


def _drive_until(primary, *fillers):
    """Round-robin emission; returns when `primary` is exhausted.
    Fillers keep their progress (pass the same generator to later phases)."""
    live = [f for f in fillers if f is not None]
    while True:
        try:
            next(primary)
        except StopIteration:
            return
        nxt = []
        for f in live:
            try:
                next(f)
                nxt.append(f)
            except StopIteration:
                pass
        live = nxt


def _drain(*gens):
    for g in gens:
        if g is None:
            continue
        for _ in g:
            pass


def build_program(use_bias):
    nc = bacc.Bacc("TRN2", target_bir_lowering=False, debug=False,
                   enable_asserts=False, num_devices=NCORES)

    def din(name, shape, dt=BF16):
        return nc.dram_tensor(name, shape, dt, kind="ExternalInput").ap()

    xT_d = din("xT", [128, KT, N], F32)
    hT_d = din("hT", [128, KT, M])
    wqk_d = din("wqk", [128, 16, KT, 128])        # q:0-7, k:8-15
    wvsa_d = din("wvsa", [128, KT, D])            # rhs layout for token-major V
    wosa_d = din("wosa", [128, 8, KT, 128])
    wqca_d = din("wqca", [128, 8, KT, 128])
    wkca_d = din("wkca", [128, 8, KT, 128])
    wvca_d = din("wvca", [128, KT, D])
    wfc_d = din("wfc", [128, 32, KT, 128])
    wproj_d = din("wproj", [128, 8, 32, 128])
    wo_ca_d = din("woca", [128, 8, KT, 128])
    any_bias = any(use_bias.values())
    if any_bias:
        bfm_d = din("bias_fm", [128, 96], F32)
        brow_d = din("bias_rows", [1, 2 * D], F32)
    outT_d = nc.dram_tensor("outT", [128, KT, N], F32,
                            kind="ExternalOutput").ap()

    from contextlib import ExitStack
    with tile.TileContext(nc) as tc, ExitStack() as ctx:
        po = {}
        po["res"] = ctx.enter_context(tc.tile_pool(name="res", bufs=1))
        po["w"] = ctx.enter_context(tc.tile_pool(name="w", bufs=3))
        po["wb"] = ctx.enter_context(tc.tile_pool(name="wb", bufs=2))
        po["small"] = ctx.enter_context(tc.tile_pool(name="small", bufs=1))
        po["work"] = ctx.enter_context(tc.tile_pool(name="work", bufs=2))
        po["stg"] = ctx.enter_context(tc.tile_pool(name="stg", bufs=3))
        po["strm"] = ctx.enter_context(tc.tile_pool(name="strm", bufs=2))
        po["ew"] = ctx.enter_context(tc.tile_pool(name="ew", bufs=4))
        po["dram"] = ctx.enter_context(
            tc.tile_pool(name="dram", bufs=1, space="DRAM"))
        po["psum_pr"] = ctx.enter_context(
            tc.tile_pool(name="psum_pr", bufs=2, space="PSUM"))
        po["psum_sc"] = ctx.enter_context(
            tc.tile_pool(name="psum_sc", bufs=2, space="PSUM"))
        po["psum_ctx"] = ctx.enter_context(
            tc.tile_pool(name="psum_ctx", bufs=2, space="PSUM"))
        po["psum_ln"] = ctx.enter_context(
            tc.tile_pool(name="psum_ln", bufs=1, space="PSUM"))

        ones32 = po["res"].tile([128, 1], BF16, tag="ones")
        nc.vector.memset(ones32[:], 1.0)
        if any_bias:
            bfm = po["res"].tile([128, 96], F32, tag="bfm")
            nc.sync.dma_start(bfm[:], bfm_d[:])
            brow = po["res"].tile([1, 2 * D], F32, tag="brow")
            nc.sync.dma_start(brow[:], brow_d[:])

        def bcol(c):
            return bfm[:, c:c + 1] if any_bias else None

        # ---- persistent SBUF state --------------------------------------
        xbuf = po["res"].tile([128, KT, N], F32, tag="xbuf")     # residual
        hbuf = po["res"].tile([128, KT, N], BF16, tag="hbuf")    # LN output
        ctxT = po["res"].tile([128, 8, N], BF16, tag="ctxT")     # attn output
        # v: [tok-in-sub(128), head, sub(8), 64 dv + 1 ones]
        v_sb = po["res"].tile([128, H, 8, 65], BF16, tag="v_sb")
        gT = po["res"].tile([128, 32, 512], BF16, tag="gT")      # MLP hidden

        nc.vector.memset(v_sb[:, :, :, 64:65], 1.0)

        for ch in range(N // 512):
            sl = slice(ch * 512, (ch + 1) * 512)
            nc.sync.dma_start(xbuf[:, :, sl], xT_d[:, :, sl])

        # DRAM scratch for q/k (contiguous tiles both ways)
        q_s = po["dram"].tile([128, 8, N], BF16, tag="q_s")      # self q
        q_c = po["dram"].tile([128, 8, N], BF16, tag="q_c")      # cross q
        k_s = po["dram"].tile([128, 8, N], BF16, tag="k_s")      # self k
        k_c = po["dram"].tile([128, 8, M], BF16, tag="k_c")      # cross k

        def vrow_bcast(col0):
            t = po["small"].tile([128, D], F32, tag="vbias")
            nc.gpsimd.partition_broadcast(t[:], brow[0:1, col0:col0 + D])
            return t

        # ---- LayerNorm (generator; yields between sub-steps) ------------
        def gen_ln(tok_sl):
            """LN of xbuf[:, :, tok_sl] (512 tokens) -> hbuf same slice."""
            t0 = tok_sl.start
            sl = slice(t0, t0 + 512)
            ps_s = po["psum_ln"].tile([1, 512], F32, tag="lns")
            ps_q = po["psum_ln"].tile([1, 512], F32, tag="lnq")
            for kt in range(KT):
                xb = po["work"].tile([128, 512], BF16, tag="xb")
                nc.vector.tensor_copy(xb[:], xbuf[:, kt, sl])
                x2c = po["work"].tile([128, 512], BF16, tag="x2c")
                nc.scalar.activation(x2c[:], xbuf[:, kt, sl], AF.Square)
                nc.tensor.matmul(ps_s[:], ones32[:], xb[:],
                                 start=(kt == 0), stop=(kt == KT - 1))
                nc.tensor.matmul(ps_q[:], ones32[:], x2c[:],
                                 start=(kt == 0), stop=(kt == KT - 1))
                if kt % 4 == 3:
                    yield
            m = po["small"].tile([1, 512], F32, tag="m")
            var = po["small"].tile([1, 512], F32, tag="var")
            rstd = po["small"].tile([1, 512], F32, tag="rstd")
            nc.vector.tensor_scalar_mul(m[:], ps_s[:], 1.0 / D)
            mm = po["small"].tile([1, 512], F32, tag="mm")
            nc.vector.tensor_tensor(mm[:], m[:], m[:], ALU.mult)
            nc.vector.scalar_tensor_tensor(var[:], ps_q[:], 1.0 / D,
                                           mm[:], ALU.mult, ALU.subtract)
            nc.vector.tensor_scalar_add(var[:], var[:], EPS)
            nc.scalar.activation(var[:], var[:], AF.Ln, bias=0.0)
            nc.scalar.activation(rstd[:], var[:], AF.Exp, scale=-0.5)
            nmrs = po["small"].tile([1, 512], F32, tag="mm")
            nc.vector.scalar_tensor_tensor(nmrs[:], m[:], -1.0, rstd[:],
                                           ALU.mult, ALU.mult)
            a_b = po["small"].tile([128, 512], F32, tag="Ab")
            b_b = po["small"].tile([128, 512], F32, tag="Bb")
            nc.gpsimd.partition_broadcast(a_b[:], rstd[0:1, :])
            nc.gpsimd.partition_broadcast(b_b[:], nmrs[0:1, :])
            yield
            for kt in range(KT):
                nc.vector.tensor_tensor(hbuf[:, kt, sl], xbuf[:, kt, sl],
                                        a_b[:], ALU.mult)
                nc.vector.tensor_tensor(hbuf[:, kt, sl], hbuf[:, kt, sl],
                                        b_b[:], ALU.add)
                if kt % 4 == 3:
                    yield

        # ---- feature-major projection (generator) -----------------------
        def gen_fm_proj(w_ap, n_ot, kt_count, rhs3, tok_sl, out_cb, wtag,
                        pool="w"):
            """for ot: psum[128,512] = sum_kt W[:,ot,kt].T @ rhs3[:,kt,tok_sl]."""
            for ot in range(n_ot):
                wst = po[pool].tile([128, kt_count, 128], BF16, tag=wtag)
                nc.sync.dma_start(wst[:], w_ap[:, ot])
                ps = po["psum_pr"].tile([128, 512], F32, tag="proj")
                for kt in range(kt_count):
                    nc.tensor.matmul(ps[:], wst[:, kt], rhs3[:, kt, tok_sl],
                                     start=(kt == 0), stop=(kt == kt_count - 1))
                    if kt == kt_count // 2:
                        yield
                out_cb(ot, ps)
                yield

        def stage_to_dram(ps, dram_ap, bias_ap):
            stg = po["stg"].tile([128, 512], BF16, tag="stg")
            if bias_ap is None:
                nc.vector.tensor_copy(stg[:], ps[:])
            else:
                nc.vector.tensor_scalar_add(stg[:], ps[:], bias_ap)
            nc.sync.dma_start(dram_ap, stg[:])

        # ---- token-major V projection (generator) -----------------------
        def gen_v_proj(h3, wv_d, sub0, tok0, vb):
            """V proj for 512 tokens [tok0, tok0+512) of h3 -> v_sb subs
            sub0..sub0+3. Layout v_sb[:, ch*8+h, sub, 0:64]."""
            for ch in range(2):           # dv chunks of 512 = 8 heads
                wvc = po["wb"].tile([128, KT, 512], BF16, tag="wbig")
                nc.sync.dma_start(wvc[:], wv_d[:, :, ch * 512:(ch + 1) * 512])
                for tt in range(4):
                    tsl = slice(tok0 + tt * 128, tok0 + (tt + 1) * 128)
                    ps = po["psum_pr"].tile([128, 512], F32, tag="proj")
                    for kt in range(KT):
                        nc.tensor.matmul(ps[:], h3[:, kt, tsl], wvc[:, kt],
                                         start=(kt == 0), stop=(kt == KT - 1))
                        if kt == KT // 2:
                            yield
                    sub = sub0 + tt
                    if vb is None:
                        nc.vector.tensor_copy(
                            v_sb[:, ch * 8:(ch + 1) * 8, sub, 0:64],
                            ps[:].rearrange("p (h e) -> p h e", e=64))
                    else:
                        nc.vector.tensor_tensor(
                            v_sb[:, ch * 8:(ch + 1) * 8, sub, 0:64],
                            ps[:].rearrange("p (h e) -> p h e", e=64),
                            vb[:, ch * 512:(ch + 1) * 512].rearrange(
                                "p (h e) -> p h e", e=64), ALU.add)
                    yield

        # ---- cross-attn K projection (generator, from hT stream) --------
        def gen_ca_k():
            for hch in range(M // 512):
                hsl = slice(hch * 512, (hch + 1) * 512)
                hTc = po["strm"].tile([128, KT, 512], BF16, tag="hTc")
                nc.sync.dma_start(hTc[:], hT_d[:, :, hsl])
                for ot in range(8):
                    wst = po["w"].tile([128, KT, 128], BF16, tag="wst8")
                    nc.sync.dma_start(wst[:], wkca_d[:, ot])
                    ps = po["psum_pr"].tile([128, 512], F32, tag="proj")
                    for kt in range(KT):
                        nc.tensor.matmul(ps[:], wst[:, kt], hTc[:, kt],
                                         start=(kt == 0), stop=(kt == KT - 1))
                        if kt % 3 == 2:
                            yield
                    bc = bcol(32 + ot) if use_bias["k_ca"] else None
                    stage_to_dram(ps, k_c[:, ot, hsl], bc)
                    yield

        # ---- cross-attn V projection (generator, from hT stream) --------
        def gen_ca_v(b):
            for hch in range(2):          # two 512-token chunks per batch
                tok0 = b * S + hch * 512
                hsl = slice(tok0, tok0 + 512)
                hTc = po["strm"].tile([128, KT, 512], BF16, tag="hTc")
                nc.sync.dma_start(hTc[:], hT_d[:, :, hsl])
                vbc = vrow_bcast(D) if use_bias["v_ca"] else None
                yield from gen_v_proj(hTc, wvca_d, 4 * hch, 0, vbc)

        # ---- attention (generator) --------------------------------------
        def gen_attention(q_dr, k_dr, sub0, s_len, b):
            """Attention for batch b: q/k strips from DRAM, v from v_sb subs
            [sub0, sub0 + s_len/128)."""
            n_s = s_len // 128
            bsl = slice(b * T, (b + 1) * T)
            for hp in range(H // 2):
                qp = po["strm"].tile([128, 512], BF16, tag="qp")
                nc.sync.dma_start(qp[:], q_dr[:, hp, bsl])
                kp = po["strm"].tile([128, 1024], BF16, tag="kp")
                nc.sync.dma_start(kp[:, 0:s_len],
                                  k_dr[:, hp, b * s_len:(b + 1) * s_len])
                ctx_e = po["psum_ctx"].tile([65, 512], F32, tag="ctx")
                ctx_o = po["psum_ctx"].tile([65, 512], F32, tag="ctx")
                h0 = hp * 2
                for c in range(n_s):
                    ssl = slice(c * 128, (c + 1) * 128)
                    sc_e = po["psum_sc"].tile([128, 512], F32, tag="sc")
                    sc_o = po["psum_sc"].tile([128, 512], F32, tag="sc")
                    # paired: rows 0-63 and 64-127 run concurrently
                    nc.tensor.matmul(sc_e[:], kp[0:64, ssl], qp[0:64, :],
                                     start=True, stop=True)
                    nc.tensor.matmul(sc_o[:], kp[64:128, ssl], qp[64:128, :],
                                     start=True, stop=True)
                    e_e = po["ew"].tile([128, 512], BF16, tag="e")
                    e_o = po["ew"].tile([128, 512], BF16, tag="e")
                    nc.scalar.activation(e_e[:], sc_e[:], AF.Exp)
                    nc.scalar.activation(e_o[:], sc_o[:], AF.Exp)
                    yield
                    nc.tensor.matmul(ctx_e[:], v_sb[:, h0, sub0 + c, :],
                                     e_e[:], start=(c == 0),
                                     stop=(c == n_s - 1))
                    nc.tensor.matmul(ctx_o[:], v_sb[:, h0 + 1, sub0 + c, :],
                                     e_o[:], start=(c == 0),
                                     stop=(c == n_s - 1))
                    yield
                # epilogue: recip of denominator row, hop to p0, bcast, mult
                for j, cx in ((0, ctx_e), (1, ctx_o)):
                    rt = po["work"].tile([65, 512], F32, tag="rt")
                    nc.vector.tensor_copy(rt[64:65, :], cx[64:65, :])
                    nc.vector.reciprocal(rt[64:65, :], rt[64:65, :])
                    ri0 = po["work"].tile([1, 512], F32, tag="ri0")
                    nc.sync.dma_start(ri0[0:1, :], rt[64:65, :])
                    rb = po["work"].tile([64, 512], F32, tag="rb")
                    nc.gpsimd.partition_broadcast(rb[:, :], ri0[0:1, :])
                    if j == 0:
                        nc.vector.tensor_tensor(ctxT[0:64, hp, bsl],
                                                cx[0:64, :], rb[:, :],
                                                ALU.mult)
                    else:
                        todd = po["work"].tile([64, 512], BF16, tag="todd")
                        nc.vector.tensor_tensor(todd[:], cx[0:64, :],
                                                rb[:, :], ALU.mult)
                        nc.sync.dma_start(ctxT[64:128, hp, bsl], todd[:])
                    yield

        # ---- out-projection (generator) ---------------------------------
        def gen_out_proj(w_d, bias_base, flag, b):
            tsl = slice(b * 512, (b + 1) * 512)

            def cb(ot, ps, _tsl=tsl):
                if flag:
                    scr = po["stg"].tile([128, 512], F32, tag="rescr")
                    nc.vector.tensor_scalar_add(scr[:], ps[:],
                                                bcol(bias_base + ot))
                    nc.vector.tensor_tensor(xbuf[:, ot, _tsl], scr[:],
                                            xbuf[:, ot, _tsl], ALU.add)
                else:
                    nc.vector.tensor_tensor(xbuf[:, ot, _tsl], ps[:],
                                            xbuf[:, ot, _tsl], ALU.add)
            yield from gen_fm_proj(w_d, 8, KT, ctxT, tsl, cb, "wst8")

        # ---- qkv for self-attention (generator) -------------------------
        def gen_sa_qkv():
            for bch in range(2):
                tsl = slice(bch * 512, (bch + 1) * 512)

                def qk_cb(ot, ps, _tsl=tsl):
                    if ot < 8:
                        bc = bcol(ot) if use_bias["qk_sa"] else None
                        stage_to_dram(ps, q_s[:, ot, _tsl], bc)
                    else:
                        o = ot - 8
                        bc = bcol(8 + o) if use_bias["qk_sa"] else None
                        stage_to_dram(ps, k_s[:, o, _tsl], bc)
                yield from gen_fm_proj(wqk_d, 16, KT, hbuf, tsl, qk_cb,
                                       "wst8")
            vb = vrow_bcast(0) if use_bias["v_sa"] else None
            for b in range(2):
                yield from gen_v_proj(hbuf, wvsa_d, 4 * b, b * 512, vb)

        # ---- cross-attn q projection (generator) ------------------------
        def gen_ca_q(b):
            tsl = slice(b * 512, (b + 1) * 512)

            def q2_cb(ot, ps, _tsl=tsl):
                bc = bcol(24 + ot) if use_bias["q_ca"] else None
                stage_to_dram(ps, q_c[:, ot, _tsl], bc)
            yield from gen_fm_proj(wqca_d, 8, KT, hbuf, tsl, q2_cb, "wst8")

        # ---- MLP (generator, one 512-token batch chunk) ------------------
        def gen_mlp(b):
            tsl = slice(b * 512, (b + 1) * 512)

            def fc_cb(ot, ps):
                sg = po["stg"].tile([128, 512], BF16, tag="sg")
                if use_bias["fc"]:
                    scr = po["stg"].tile([128, 512], F32, tag="rescr")
                    nc.vector.tensor_scalar_add(scr[:], ps[:], bcol(48 + ot))
                    nc.scalar.activation(sg[:], scr[:], AF.Sigmoid,
                                         scale=GELU_A)
                    nc.vector.tensor_tensor(gT[:, ot], scr[:], sg[:],
                                            ALU.mult)
                else:
                    nc.scalar.activation(sg[:], ps[:], AF.Sigmoid,
                                         scale=GELU_A)
                    nc.vector.tensor_tensor(gT[:, ot], ps[:], sg[:],
                                            ALU.mult)
            yield from gen_fm_proj(wfc_d, 32, KT, hbuf, tsl, fc_cb, "wst8")

            def proj_cb(ot, ps, _tsl=tsl):
                if use_bias["proj"]:
                    scr = po["stg"].tile([128, 512], F32, tag="rescr")
                    nc.vector.tensor_scalar_add(scr[:], ps[:], bcol(88 + ot))
                    nc.vector.tensor_tensor(xbuf[:, ot, _tsl], scr[:],
                                            xbuf[:, ot, _tsl], ALU.add)
                else:
                    nc.vector.tensor_tensor(xbuf[:, ot, _tsl], ps[:],
                                            xbuf[:, ot, _tsl], ALU.add)
                nc.sync.dma_start(outT_d[:, ot, _tsl], xbuf[:, ot, _tsl])
            yield from gen_fm_proj(wproj_d, 8, 32, gT, slice(0, 512),
                                   proj_cb, "wbig", pool="wb")

        # =================== schedule ====================================
        cak = gen_ca_k()

        # P0: LN1 zipped with cross-K (independent, fills the LN ramp)
        _drive_until(gen_ln(slice(0, 512)), cak)
        _drive_until(gen_ln(slice(512, 1024)), cak)

        # P1: SA qkv (dense) zipped lightly with remaining cross-K
        _drive_until(gen_sa_qkv(), cak)

        # P2: SA attention; b1 zipped with SAout(b0)+LN2(b0)+CAq(b0)
        _drive_until(gen_attention(q_s, k_s, 0, T, 0), cak)

        def gen_tail0():
            yield from gen_out_proj(wosa_d, 16, use_bias["o_sa"], 0)
            yield from gen_ln(slice(0, 512))
            yield from gen_ca_q(0)
        tail0 = gen_tail0()
        _drive_until(gen_attention(q_s, k_s, 4, T, 1), tail0, cak)

        # P3: SAout(b1) + LN2(b1) + CAq(b1) + CA-V(b0)  (dense)
        def gen_tail1():
            yield from gen_out_proj(wosa_d, 16, use_bias["o_sa"], 1)
            yield from gen_ln(slice(512, 1024))
            yield from gen_ca_q(1)
        _drain(tail0, cak)
        _drive_until(gen_tail1(), gen_ca_v(0))

        # P4: CA attention b0 (exp-bound; nothing independent left)
        _drive_until(gen_attention(q_c, k_c, 0, S, 0))

        # P4.5/P5: CA-V(b1), then CA attention b1, zipped with
        # CAout(b0)+LN3(b0)+MLP(b0)
        def gen_tail2():
            yield from gen_out_proj(wo_ca_d, 40, use_bias["o_ca"], 0)
            yield from gen_ln(slice(0, 512))
            yield from gen_mlp(0)
        tail2 = gen_tail2()
        _drive_until(gen_ca_v(1), tail2)
        _drive_until(gen_attention(q_c, k_c, 0, S, 1), tail2)

        # P6: CAout(b1) + LN3(b1) + MLP(b1)  (dense)
        def gen_tail3():
            yield from gen_out_proj(wo_ca_d, 40, use_bias["o_ca"], 1)
            yield from gen_ln(slice(512, 1024))
            yield from gen_mlp(1)
        _drive_until(gen_tail3(), tail2)

    nc.compile()
    return nc


# ---------------------------------------------------------------------------
# host side
# ---------------------------------------------------------------------------

def _tile4(w):
    """[Din, Dout] -> [128, Dout/128, Din/128, 128] (p, ot, kt, o)."""
    din, dout = w.shape
    return np.ascontiguousarray(
        w.reshape(din // 128, 128, dout // 128, 128).transpose(1, 2, 0, 3))


def _rhs_tiled(w):
    """[Din, Dout] -> [128, Din/128, Dout] (p, kt, o)."""
    din, dout = w.shape
    return np.ascontiguousarray(
        w.reshape(din // 128, 128, dout).transpose(1, 0, 2))


def _fm_cols(b):
    """[Dout] -> [128, Dout/128] (p, ot)."""
    return np.ascontiguousarray(b.reshape(-1, 128).T)


def _prep_host(inputs):
    f32 = np.float32
    g = {k: np.asarray(v, f32) for k, v in inputs.items()}
    x, hs = g["x"], g["hidden_states"]
    scale = f32(1.0 / np.sqrt(HD))

    wq, wk, wv = np.split(g["sa_in_w"], 3, axis=0)
    bq, bk, bv = np.split(g["sa_in_b"], 3)
    wq_e = (wq * g["ln1_g"][None, :]) * scale
    bq_e = (wq @ g["ln1_b"]) * scale + bq
    wk_e = wk * g["ln1_g"][None, :]
    bk_e = wk @ g["ln1_b"] + bk
    wv_e = wv * g["ln1_g"][None, :]
    bv_e = wv @ g["ln1_b"] + bv

    cq, ck, cv = np.split(g["ca_in_w"], 3, axis=0)
    cbq, cbk, cbv = np.split(g["ca_in_b"], 3)
    cq_e = (cq * g["ln2_g"][None, :]) * scale
    cbq_e = (cq @ g["ln2_b"]) * scale + cbq
    # k/v of cross-attn apply to raw hidden_states: no LN fold
    fc_e = g["fc_w"] * g["ln3_g"][None, :]
    fcb_e = g["fc_w"] @ g["ln3_b"] + g["fc_b"]

    wqk = np.concatenate([wq_e, wk_e], axis=0)     # [2D, D]
    nz = lambda a: bool(np.abs(a).max() > 0)
    use_bias = dict(
        qk_sa=nz(np.concatenate([bq_e, bk_e])), v_sa=nz(bv_e),
        o_sa=nz(g["sa_out_b"]), q_ca=nz(cbq_e), k_ca=nz(cbk), v_ca=nz(cbv),
        o_ca=nz(g["ca_out_b"]), fc=nz(fcb_e), proj=nz(g["proj_b"]),
    )

    bf = lambda a: np.ascontiguousarray(a.astype(BF16NP))
    weights = {
        "wqk": bf(_tile4(wqk.T)),
        "wvsa": bf(_rhs_tiled(wv_e.T)),
        "wosa": bf(_tile4(g["sa_out_w"].T)),
        "wqca": bf(_tile4(cq_e.T)),
        "wkca": bf(_tile4(ck.T)),
        "wvca": bf(_rhs_tiled(cv.T)),
        "woca": bf(_tile4(g["ca_out_w"].T)),
        "wfc": bf(_tile4(fc_e.T)),
        "wproj": bf(_tile4(g["proj_w"].T)),
    }
    if any(use_bias.values()):
        bfm = np.zeros((128, 96), f32)
        bfm[:, 0:8] = _fm_cols(bq_e)
        bfm[:, 8:16] = _fm_cols(bk_e)
        bfm[:, 16:24] = _fm_cols(g["sa_out_b"])
        bfm[:, 24:32] = _fm_cols(cbq_e)
        bfm[:, 32:40] = _fm_cols(cbk)
        bfm[:, 40:48] = _fm_cols(g["ca_out_b"])
        bfm[:, 48:80] = _fm_cols(fcb_e)
        bfm[:, 88:96] = _fm_cols(g["proj_b"])
        brow = np.zeros((1, 2 * D), f32)
        brow[0, 0:D] = bv_e
        brow[0, D:2 * D] = cbv
        weights["bias_fm"] = bfm
        weights["bias_rows"] = brow

    in_maps = []
    for c in range(NCORES):
        xs = x[:, 2 * c:2 * c + 2, :]              # [T, 2, D]
        xt = xs.transpose(2, 1, 0).reshape(KT, 128, N).transpose(1, 0, 2)
        hss = hs[:, 2 * c:2 * c + 2, :]
        ht = hss.transpose(2, 1, 0).reshape(KT, 128, M).transpose(1, 0, 2)
        im = dict(weights)
        im["xT"] = np.ascontiguousarray(xt.astype(f32))
        im["hT"] = bf(ht)
        in_maps.append(im)
    return in_maps, use_bias


def _unshard(results):
    out = np.empty((T, B, D), np.float32)
    for c in range(NCORES):
        r = np.asarray(results[c]["outT"])         # [128, KT, N]
        arr = r.transpose(1, 0, 2).reshape(D, BPC, T)
        out[:, 2 * c:2 * c + 2, :] = arr.transpose(2, 1, 0)
    return out


_cache = {}


def _get_program(key):
    if key not in _cache:
        _cache[key] = build_program(dict(key))
    return _cache[key]


def kernel(**inputs):
    in_maps, use_bias = _prep_host(inputs)
    nc = _get_program(tuple(sorted(use_bias.items())))
    res = bass_utils.run_bass_kernel_spmd(nc, in_maps,
                                          core_ids=list(range(NCORES)))
    return _unshard(res.results)


def kernel_traced(**inputs):
    """Like kernel() but with NTFF profiling; returns (out, exec_time_ns)."""
    import types
    import antenv  # noqa: F401
    if "antenv.axon_hooks" not in sys.modules:
        hooks = types.ModuleType("antenv.axon_hooks")
        hooks._hook = None
        hooks.set_axon_ntff_profile_hook = lambda h: setattr(hooks, "_hook", h)
        hooks.get_axon_ntff_profile_hook = lambda: hooks._hook
        sys.modules["antenv.axon_hooks"] = hooks
        try:
            import trn_agent_boot.trn_boot as _tb
            hooks._hook = _tb._ntff_profile_via_ctypes("/opt/axon/libaxon_pjrt.so")
        except Exception as e:  # pragma: no cover
            print("ntff hook unavailable:", e)
    in_maps, use_bias = _prep_host(inputs)
    nc = _get_program(tuple(sorted(use_bias.items())))
    res = bass_utils.run_bass_kernel_spmd(nc, in_maps,
                                          core_ids=list(range(NCORES)),
                                          trace=True)
    return _unshard(res.results), res.exec_time_ns
